# revision 22
# baseline (speedup 1.0000x reference)
"""Trainium2 Bass kernel: AttentionFlow layer (BiDAF-style), data-parallel over batch.

Reference semantics (per batch b, shapes C[Tc,d], Q[Tq,d], w[3d]):
    w1, w2, w3 = w[:d], w[d:2d], w[2d:]
    S[t,q]  = C[t].w1 + Q[q].w2 + (C[t]*w3).Q[q]
    P       = softmax_q(S)
    bt      = softmax_t(max_q S)
    U       = P @ Q
    h       = bt @ C
    G       = concat(C, U, C*U, C*h[None,:])   # [Tc, 4d]

On-chip identities used:
  - softmax_q(S) drops the C.w1 term (constant along q):  P = E/Z with
    E = exp(dot + q2), dot[t,q] = (C*w3)[t].Q[q], q2[q] = Q[q].w2.
    |dot + q2| <~ 5 so exp is fp32-safe without max subtraction.
  - max_q S = c1 + max_q(dot + q2) with c1 = C.w1 (extra w1 column on the
    S-matmul rhs; q2 row added with a K=1 ones-row matmul). S is only used
    for the row-max.
  - E^T = exp(S'^T + q2) with S'^T = qta^T @ C^T computed per t-tile PAIR;
    the q2 add rides the exp's per-partition bias operand (q on partitions
    in this orientation), saving the ones-rhs matmuls.
  - [U_raw | Z] = E @ [Q | 1]  (ones column appended to Q).
  - h_raw accumulated per tile; Zb via a ones-lhsT matmul over e2 + reduce.
  - Matmuls run as float32r; f32r SBUF operands must be PRODUCED as f32r.

Engine placement (keeps the DMA queues fed and compute queues unblocked):
  - out G[:, 0:3d] per tile -> SP-issued HWDGE DMA (SP is otherwise idle, so
    its trigger waits don't block compute).
  - out G[:, 3d:4d] (C*h) per tile: multiplied on GpSimd and DMA'd from
    GpSimd right after -- the trigger's wait is same-engine, i.e. free.
  - C/Q loads are software-prefetched one batch ahead on GpSimd.
  - PSUM->SBUF C^T copies alternate Scalar/Vector; U-mul on Scalar,
    C*U on Vector, row-max chain on Vector.

DMA layout: p-major row mapping (row = p*16 + s for C/G, row = 2p + s for
Q): the C load is one 2 MB DMA with 16 KB/partition descriptors; per-tile G
writes have one contiguous 3 KB (resp. 1 KB) descriptor per partition. The
t/q permutation is internal: all math is row-local or full reductions, so
the same mapping on loads and stores cancels it.
"""

import numpy as np

import concourse.bass as bass
import concourse.bacc as bacc
import concourse.mybir as mybir
import concourse.tile as tile
from contextlib import ExitStack
from concourse.masks import make_identity

F32 = mybir.dt.float32
F32R = mybir.dt.float32r
AX = mybir.AxisListType
AF = mybir.ActivationFunctionType

B, TC, TQ, D = 32, 2048, 256, 256
N_CORES = 8
BPC = B // N_CORES


def _f32(ap):
    """Plain-fp32 view of a float32r tile for non-matmul readers."""
    return ap.bitcast(F32)


def build_nc(bpc=BPC, tcl=TC, tq=TQ, d=D, reps=None, emit_out=True):
    nt = tcl // 128  # t-tiles per batch; tile s holds rows {p*nt + s}
    nd = d // 128    # K-chunks over d
    nq = tq // 128   # K-chunks over q
    npair = nt // 2
    assert nt % 2 == 0

    nc = bacc.Bacc(None, debug=False, target_bir_lowering=False)
    c_in = nc.declare_dram_parameter("context_emb", [bpc, tcl, d], F32, isOutput=False)
    q_in = nc.declare_dram_parameter("query_emb", [bpc, tq, d], F32, isOutput=False)
    w_in = nc.declare_dram_parameter("w", [3 * d], F32, isOutput=False)
    out_e = nc.declare_dram_parameter("out", [bpc, tcl, 4 * d], F32, isOutput=True)

    with tile.TileContext(nc) as tc, ExitStack() as ctx:
        singles = ctx.enter_context(tc.tile_pool(name="singles", bufs=1))
        cb_pool = ctx.enter_context(tc.tile_pool(name="cb", bufs=2))
        qb_pool = ctx.enter_context(tc.tile_pool(name="qb", bufs=2))
        pb_pool = ctx.enter_context(tc.tile_pool(name="pb", bufs=2))
        ct_pool = ctx.enter_context(tc.tile_pool(name="ct", bufs=4))
        et_pool = ctx.enter_context(tc.tile_pool(name="et", bufs=4))
        gu_pool = ctx.enter_context(tc.tile_pool(name="gu", bufs=8))
        g4_pool = ctx.enter_context(tc.tile_pool(name="g4", bufs=8))
        sm_pool = ctx.enter_context(tc.tile_pool(name="sm", bufs=6))
        ps2k = ctx.enter_context(tc.tile_pool(name="ps2k", bufs=2, space="PSUM"))
        psH = ctx.enter_context(tc.tile_pool(name="psH", bufs=1, space="PSUM"))
        psU = ctx.enter_context(tc.tile_pool(name="psU", bufs=2, space="PSUM"))
        psC = ctx.enter_context(tc.tile_pool(name="psC", bufs=2, space="PSUM"))

        ident = singles.tile([128, 128], F32, tag="ident")
        make_identity(nc, ident)
        onesf_col = singles.tile([128, 8], F32, tag="onesf_col")
        nc.vector.memset(onesf_col, 1.0)
        # oz[:, s, :] = [1.0, 0.0] -- pad columns for the even-N f32r matmuls
        oz = singles.tile([128, 8, 2], F32, tag="oz")
        nc.vector.memset(oz[:, :, 0:1], 1.0)
        nc.vector.memset(oz[:, :, 1:2], 0.0)
        zerof_col = singles.tile([128, 1], F32, tag="zerof_col")
        nc.vector.memset(zerof_col, 0.0)
        onesf_row = singles.tile([1, 256], F32, tag="onesf_row")
        nc.vector.memset(onesf_row, 1.0)
        zerof = singles.tile([1, 1], F32, tag="zerof")
        nc.vector.memset(zerof, 0.0)
        ones128 = singles.tile([1, 128], F32R, tag="ones128")
        nc.vector.tensor_copy(out=ones128, in_=onesf_row[:, 0:128])
        ones2r = singles.tile([128, 2], F32R, tag="ones2r")
        nc.vector.tensor_copy(out=ones2r, in_=onesf_col[:, 0:2])
        # wcols[p, k] = w[k*128 + p]: chunk columns [w1 | w2 | w3].
        # w comes in as one contiguous row (single descriptor) and is spread
        # onto partitions with K=1 matmuls.
        wrow = singles.tile([1, 3 * d], F32R, tag="wrow")
        nc.gpsimd.dma_start(out=wrow, in_=w_in[:].rearrange("(a w) -> a w", a=1).bitcast(F32R))
        wcols = singles.tile([128, 3 * nd], F32R, tag="wcols")
        pswc = psC.tile([128, 2 * 3 * nd], F32, tag="psC")
        for k in range(3 * nd):
            nc.tensor.matmul(
                pswc[:, 2 * k : 2 * k + 2],
                wrow[:, k * 128 : (k + 1) * 128],
                ones128[:, 0:2],
                start=True,
                stop=True,
            )
        for k in range(3 * nd):
            nc.vector.tensor_copy(
                out=wcols[:, k : k + 1], in_=pswc[:, 2 * k : 2 * k + 1]
            )
        # w1z[:, dj, :] = [w1 chunk | 0] -- N=2 rhs for the c1 matmuls
        w1z = singles.tile([128, nd, 2], F32R, tag="w1z")
        for dj in range(nd):
            nc.vector.tensor_copy(out=w1z[:, dj, 0:1], in_=_f32(wcols[:, dj : dj + 1]))
            nc.vector.tensor_copy(out=w1z[:, dj, 1:2], in_=zerof_col)

        def load(b):
            qaug = qb_pool.tile([128, nq, d + 2], F32R, tag="qaug", name="qaug")
            nc.gpsimd.dma_start(
                out=qaug[:, :, 0:d],
                in_=q_in[b].rearrange("(p s) d -> p s d", p=128).bitcast(F32R),
            )
            cb = cb_pool.tile([128, nt, d], F32R, tag="cb", name="cb")
            cv = c_in[b].rearrange("(p s) d -> p s d", p=128).bitcast(F32R)
            for g in range(4):
                sl = slice(g * (nt // 4), (g + 1) * (nt // 4))
                nc.gpsimd.dma_start(out=cb[:, sl, :], in_=cv[:, sl, :])
            return cb, qaug

        def compute(b, cb, qaug):
            # ---- per-batch Q prep ----
            nc.vector.tensor_copy(out=qaug[:, :, d : d + 2], in_=oz[:, 0:nq, :])

            qt = qb_pool.tile([128, nd, tq], F32R, tag="qt")
            psq = psC.tile([128, nd * tq], F32, tag="psC")
            for dj in range(nd):
                for qi in range(nq):
                    nc.tensor.transpose(
                        psq[:, dj * tq + qi * 128 : dj * tq + (qi + 1) * 128],
                        _f32(qaug[:, qi, dj * 128 : (dj + 1) * 128]),
                        ident,
                    )
            nc.scalar.copy(out=qt, in_=psq)

            # q2 row = w2^T @ Q^T -> [1, tq]; pad col tq with 0
            psq2 = psU.tile([1, tq], F32, tag="psU")
            for dj in range(nd):
                nc.tensor.matmul(
                    psq2,
                    wcols[:, nd + dj : nd + dj + 1],
                    qt[:, dj, :],
                    start=(dj == 0),
                    stop=(dj == nd - 1),
                )
            q2row = pb_pool.tile([1, tq], F32R, tag="q2row")
            nc.vector.tensor_copy(out=q2row, in_=psq2)

            # q2col[p, qi] = q2[2p+qi]  (bias operand for the E^T exp);
            # a K=1 matmul broadcasts the q2 row chunk onto partitions
            psqc = psC.tile([128, 2 * nq], F32, tag="psC")
            for qi in range(nq):
                nc.tensor.matmul(
                    psqc[:, qi * 2 : (qi + 1) * 2],
                    q2row[:, qi * 128 : (qi + 1) * 128],
                    ones128[:, 0:2],
                    start=True,
                    stop=True,
                )
            q2col = pb_pool.tile([128, nq], F32, tag="q2col")
            for qi in range(nq):
                nc.vector.tensor_copy(
                    out=q2col[:, qi : qi + 1], in_=psqc[:, qi * 2 : qi * 2 + 1]
                )

            # qta[:, dj, :] = w3-scaled Q^T chunk
            qta = qb_pool.tile([128, nd, tq], F32R, tag="qta")
            for dj in range(nd):
                nc.vector.tensor_scalar_mul(
                    out=qta[:, dj, :],
                    in0=_f32(qt[:, dj, :]),
                    scalar1=_f32(wcols[:, 2 * nd + dj : 2 * nd + dj + 1]),
                )

            mfull = pb_pool.tile([128, nt], F32, tag="mfull")
            e2 = pb_pool.tile([128, nt], F32R, tag="e2")
            psh = psH.tile([1, d], F32, tag="psH")
            psc1 = psU.tile([128, nt, 2], F32, tag="psz", bufs=1)
            outv = out_e[b].rearrange("(p s) d -> p s d", p=128)
            # G's first d columns are C verbatim: one 2 MB DMA straight from
            # cb, queued at batch start so the out direction is busy from t=0
            if emit_out:
                nc.sync.dma_start(out=outv[:, :, 0:d], in_=_f32(cb))

            # ---- phase A: t-tile pairs ----
            for pj in range(npair):
                psc2 = psC.tile([128, nd * 256], F32, tag="psC")
                for jj in range(2):
                    s = 2 * pj + jj
                    for dj in range(nd):
                        nc.tensor.transpose(
                            psc2[:, dj * 256 + jj * 128 : dj * 256 + (jj + 1) * 128],
                            _f32(cb[:, s, dj * 128 : (dj + 1) * 128]),
                            ident,
                        )
                ct2 = ct_pool.tile([128, nd * 256], F32R, tag="ct2")
                if pj % 2 == 0:
                    nc.scalar.copy(out=ct2, in_=psc2)
                else:
                    nc.vector.tensor_copy(out=ct2, in_=psc2)

                # S for the PAIR in one bank [128, 2, tq] (row-max only);
                # c1 = C.w1 accumulates separately into psc1
                pss = ps2k.tile([128, 2, tq], F32, tag="ps2k")
                for jj in range(2):
                    s = 2 * pj + jj
                    for dj in range(nd):
                        nc.tensor.matmul(
                            pss[:, jj, :],
                            ct2[:, dj * 256 + jj * 128 : dj * 256 + (jj + 1) * 128],
                            qta[:, dj, :],
                            start=(jj == 0 and dj == 0),
                            stop=False,
                        )
                    nc.tensor.matmul(
                        pss[:, jj, :], ones128, q2row, start=False, stop=(jj == 1)
                    )
                    for dj in range(nd):
                        nc.tensor.matmul(
                            psc1[:, s, :],
                            ct2[:, dj * 256 + jj * 128 : dj * 256 + (jj + 1) * 128],
                            w1z[:, dj, :],
                            start=(dj == 0),
                            stop=(dj == nd - 1),
                        )
                nc.vector.reduce_max(
                    out=mfull[:, 2 * pj : 2 * pj + 2], in_=pss, axis=AX.X
                )

                # S'^T for the pair: psT2 layout [qi, (jj t)]; q2 added via
                # the exp bias (q on partitions here)
                psT2 = ps2k.tile([128, nq * 256], F32, tag="ps2k")
                for qi in range(nq):
                    sl = slice(qi * 256, (qi + 1) * 256)
                    for dj in range(nd):
                        nc.tensor.matmul(
                            psT2[:, sl],
                            qta[:, dj, qi * 128 : (qi + 1) * 128],
                            ct2[:, dj * 256 : (dj + 1) * 256],
                            start=(dj == 0),
                            stop=(dj == nd - 1),
                        )
                et2 = et_pool.tile([128, nq * 256], F32R, tag="et2")
                for qi in range(nq):
                    sl = slice(qi * 256, (qi + 1) * 256)
                    nc.scalar.activation(
                        out=et2[:, sl],
                        in_=psT2[:, sl],
                        func=AF.Exp,
                        bias=q2col[:, qi : qi + 1],
                    )

                # [U_raw | Z] = E @ [Q | 1]; stream [C | U | C*U] out per tile
                for jj in range(2):
                    s = 2 * pj + jj
                    psu = psU.tile([128, d + 2], F32, tag="psU")
                    for qi in range(nq):
                        nc.tensor.matmul(
                            psu,
                            et2[:, qi * 256 + jj * 128 : qi * 256 + (jj + 1) * 128],
                            qaug[:, qi, :],
                            start=(qi == 0),
                            stop=(qi == nq - 1),
                        )
                    rz = sm_pool.tile([128, 1], F32, tag="rz")
                    nc.vector.reciprocal(out=rz, in_=psu[:, d : d + 1])
                    gu = gu_pool.tile([128, 2 * d], F32, tag="gu")
                    nc.scalar.mul(gu[:, 0:d], psu[:, 0:d], rz)
                    nc.vector.tensor_mul(
                        out=gu[:, d : 2 * d],
                        in0=_f32(cb[:, s, :]),
                        in1=gu[:, 0:d],
                    )
                    if emit_out:
                        nc.sync.dma_start(out=outv[:, s, d : 3 * d], in_=gu)

            # ---- phase A epilogue: m = rowmax + c1, e2, h accumulation ----
            nc.vector.tensor_add(out=mfull, in0=mfull, in1=psc1[:, :, 0])
            nc.scalar.activation(out=e2, in_=mfull, func=AF.Exp)
            for s in range(nt):
                nc.tensor.matmul(
                    psh,
                    e2[:, s : s + 1],
                    cb[:, s, :],
                    start=(s == 0),
                    stop=(s == nt - 1),
                )

            # ---- phase B: Zb, h, hb; C*h multiplied and DMA'd on GpSimd ----
            psz = psU.tile([1, nt], F32, tag="psz", bufs=1)
            nc.tensor.matmul(psz, ones2r[:, 0:1], e2, start=True, stop=True)
            zsum = sm_pool.tile([1, 1], F32, tag="zsum")
            nc.vector.reduce_sum(out=zsum, in_=psz, axis=AX.X)
            zb = sm_pool.tile([1, 1], F32, tag="zb")
            nc.vector.reciprocal(out=zb, in_=zsum)
            hrow = pb_pool.tile([1, d], F32R, tag="hrow")
            nc.vector.tensor_scalar_mul(out=hrow, in0=psh, scalar1=zb)
            pshb = psU.tile([128, d], F32, tag="psz", bufs=1)
            nc.tensor.matmul(pshb, ones128, hrow, start=True, stop=True)
            hb = pb_pool.tile([128, d], F32, tag="hb")
            nc.scalar.copy(out=hb, in_=pshb)
            for s in range(nt):
                g4 = g4_pool.tile([128, d], F32, tag="g4")
                if s % 2 == 0:
                    nc.gpsimd.tensor_mul(out=g4, in0=_f32(cb[:, s, :]), in1=hb)
                    if emit_out:
                        nc.gpsimd.dma_start(out=outv[:, s, 3 * d : 4 * d], in_=g4)
                else:
                    nc.vector.tensor_mul(out=g4, in0=_f32(cb[:, s, :]), in1=hb)
                    if emit_out:
                        nc.sync.dma_start(out=outv[:, s, 3 * d : 4 * d], in_=g4)

        def body():
            tiles = load(0)
            for b in range(bpc):
                nxt = load(b + 1) if b + 1 < bpc else None
                compute(b, *tiles)
                tiles = nxt

        if reps is None:
            body()
        else:
            with tc.For_i(0, reps, 1):
                body()

    return nc


_NC_CACHE = {}


def _get_nc(bpc=BPC, tcl=TC, tq=TQ, d=D):
    key = (bpc, tcl, tq, d)
    if key not in _NC_CACHE:
        _NC_CACHE[key] = build_nc(*key)
    return _NC_CACHE[key]


def _run(context_emb, query_emb, w, trace=False, **spmd_kwargs):
    from concourse.bass_utils import run_bass_kernel_spmd

    context_emb = np.ascontiguousarray(np.asarray(context_emb, dtype=np.float32))
    query_emb = np.ascontiguousarray(np.asarray(query_emb, dtype=np.float32))
    w = np.ascontiguousarray(np.asarray(w, dtype=np.float32))

    nc = _get_nc()
    if not nc.is_finalized():
        nc.finalize()
    in_maps = []
    for c in range(N_CORES):
        sl = slice(c * BPC, (c + 1) * BPC)
        in_maps.append(
            {
                "context_emb": np.ascontiguousarray(context_emb[sl]),
                "query_emb": np.ascontiguousarray(query_emb[sl]),
                "w": w,
            }
        )
    res = run_bass_kernel_spmd(
        nc, in_maps, core_ids=list(range(N_CORES)), trace=trace, **spmd_kwargs
    )
    out = np.concatenate([r["out"] for r in res.results], axis=0)
    return out, res


def kernel(context_emb, query_emb, w):
    out, _ = _run(context_emb, query_emb, w, trace=False)
    return out


# revision 25
# speedup vs baseline: 1.0057x; 1.0057x over previous
"""Trainium2 Bass kernel: AttentionFlow layer (BiDAF-style), data-parallel over batch.

Reference semantics (per batch b, shapes C[Tc,d], Q[Tq,d], w[3d]):
    w1, w2, w3 = w[:d], w[d:2d], w[2d:]
    S[t,q]  = C[t].w1 + Q[q].w2 + (C[t]*w3).Q[q]
    P       = softmax_q(S)
    bt      = softmax_t(max_q S)
    U       = P @ Q
    h       = bt @ C
    G       = concat(C, U, C*U, C*h[None,:])   # [Tc, 4d]

On-chip identities used:
  - softmax_q(S) drops the C.w1 term (constant along q):  P = E/Z with
    E = exp(dot + q2), dot[t,q] = (C*w3)[t].Q[q], q2[q] = Q[q].w2.
    |dot + q2| <~ 5 so exp is fp32-safe without max subtraction.
  - max_q S = c1 + max_q(dot + q2) with c1 = C.w1 (extra w1 column on the
    S-matmul rhs; q2 row added with a K=1 ones-row matmul). S is only used
    for the row-max.
  - E^T = exp(S'^T + q2) with S'^T = qta^T @ C^T computed per t-tile PAIR;
    the q2 add rides the exp's per-partition bias operand (q on partitions
    in this orientation), saving the ones-rhs matmuls.
  - [U_raw | Z] = E @ [Q | 1]  (ones column appended to Q).
  - h_raw accumulated per tile; Zb via a ones-lhsT matmul over e2 + reduce.
  - Matmuls run as float32r; f32r SBUF operands must be PRODUCED as f32r.

Engine placement (keeps the DMA queues fed and compute queues unblocked):
  - out G[:, 0:3d] per tile -> SP-issued HWDGE DMA (SP is otherwise idle, so
    its trigger waits don't block compute).
  - out G[:, 3d:4d] (C*h) per tile: multiplied on GpSimd and DMA'd from
    GpSimd right after -- the trigger's wait is same-engine, i.e. free.
  - C/Q loads are software-prefetched one batch ahead on GpSimd.
  - PSUM->SBUF C^T copies alternate Scalar/Vector; U-mul on Scalar,
    C*U on Vector, row-max chain on Vector.

DMA layout: p-major row mapping (row = p*16 + s for C/G, row = 2p + s for
Q): the C load is one 2 MB DMA with 16 KB/partition descriptors; per-tile G
writes have one contiguous 3 KB (resp. 1 KB) descriptor per partition. The
t/q permutation is internal: all math is row-local or full reductions, so
the same mapping on loads and stores cancels it.
"""

import numpy as np

import concourse.bass as bass
import concourse.bacc as bacc
import concourse.mybir as mybir
import concourse.tile as tile
from contextlib import ExitStack
from concourse.masks import make_identity

F32 = mybir.dt.float32
F32R = mybir.dt.float32r
AX = mybir.AxisListType
AF = mybir.ActivationFunctionType

B, TC, TQ, D = 32, 2048, 256, 256
N_CORES = 8
BPC = B // N_CORES


def _f32(ap):
    """Plain-fp32 view of a float32r tile for non-matmul readers."""
    return ap.bitcast(F32)


def build_nc(bpc=BPC, tcl=TC, tq=TQ, d=D, reps=None, emit_out=True):
    nt = tcl // 128  # t-tiles per batch; tile s holds rows {p*nt + s}
    nd = d // 128    # K-chunks over d
    nq = tq // 128   # K-chunks over q
    npair = nt // 2
    assert nt % 2 == 0

    nc = bacc.Bacc(None, debug=False, target_bir_lowering=False)
    c_in = nc.declare_dram_parameter("context_emb", [bpc, tcl, d], F32, isOutput=False)
    q_in = nc.declare_dram_parameter("query_emb", [bpc, tq, d], F32, isOutput=False)
    w_in = nc.declare_dram_parameter("w", [3 * d], F32, isOutput=False)
    out_e = nc.declare_dram_parameter("out", [bpc, tcl, 4 * d], F32, isOutput=True)

    with tile.TileContext(nc) as tc, ExitStack() as ctx:
        singles = ctx.enter_context(tc.tile_pool(name="singles", bufs=1))
        cb_pool = ctx.enter_context(tc.tile_pool(name="cb", bufs=3))
        qb_pool = ctx.enter_context(tc.tile_pool(name="qb", bufs=3))
        pb_pool = ctx.enter_context(tc.tile_pool(name="pb", bufs=2))
        ct_pool = ctx.enter_context(tc.tile_pool(name="ct", bufs=4))
        et_pool = ctx.enter_context(tc.tile_pool(name="et", bufs=4))
        gu_pool = ctx.enter_context(tc.tile_pool(name="gu", bufs=8))
        g4_pool = ctx.enter_context(tc.tile_pool(name="g4", bufs=8))
        sm_pool = ctx.enter_context(tc.tile_pool(name="sm", bufs=6))
        ps2k = ctx.enter_context(tc.tile_pool(name="ps2k", bufs=2, space="PSUM"))
        psH = ctx.enter_context(tc.tile_pool(name="psH", bufs=1, space="PSUM"))
        psU = ctx.enter_context(tc.tile_pool(name="psU", bufs=2, space="PSUM"))
        psC = ctx.enter_context(tc.tile_pool(name="psC", bufs=2, space="PSUM"))

        ident = singles.tile([128, 128], F32, tag="ident")
        make_identity(nc, ident)
        onesf_col = singles.tile([128, 8], F32, tag="onesf_col")
        nc.vector.memset(onesf_col, 1.0)
        # oz[:, s, :] = [1.0, 0.0] -- pad columns for the even-N f32r matmuls
        oz = singles.tile([128, 8, 2], F32, tag="oz")
        nc.vector.memset(oz[:, :, 0:1], 1.0)
        nc.vector.memset(oz[:, :, 1:2], 0.0)
        zerof_col = singles.tile([128, 1], F32, tag="zerof_col")
        nc.vector.memset(zerof_col, 0.0)
        onesf_row = singles.tile([1, 256], F32, tag="onesf_row")
        nc.vector.memset(onesf_row, 1.0)
        zerof = singles.tile([1, 1], F32, tag="zerof")
        nc.vector.memset(zerof, 0.0)
        ones128 = singles.tile([1, 128], F32R, tag="ones128")
        nc.vector.tensor_copy(out=ones128, in_=onesf_row[:, 0:128])
        ones2r = singles.tile([128, 2], F32R, tag="ones2r")
        nc.vector.tensor_copy(out=ones2r, in_=onesf_col[:, 0:2])
        # wcols[p, k] = w[k*128 + p]: chunk columns [w1 | w2 | w3].
        # w comes in as one contiguous row (single descriptor) and is spread
        # onto partitions with K=1 matmuls.
        wrow = singles.tile([1, 3 * d], F32R, tag="wrow")
        nc.gpsimd.dma_start(out=wrow, in_=w_in[:].rearrange("(a w) -> a w", a=1).bitcast(F32R))
        wcols = singles.tile([128, 3 * nd], F32R, tag="wcols")
        pswc = psC.tile([128, 2 * 3 * nd], F32, tag="psC")
        for k in range(3 * nd):
            nc.tensor.matmul(
                pswc[:, 2 * k : 2 * k + 2],
                wrow[:, k * 128 : (k + 1) * 128],
                ones128[:, 0:2],
                start=True,
                stop=True,
            )
        for k in range(3 * nd):
            nc.vector.tensor_copy(
                out=wcols[:, k : k + 1], in_=pswc[:, 2 * k : 2 * k + 1]
            )
        # w1z[:, dj, :] = [w1 chunk | 0] -- N=2 rhs for the c1 matmuls
        w1z = singles.tile([128, nd, 2], F32R, tag="w1z")
        for dj in range(nd):
            nc.vector.tensor_copy(out=w1z[:, dj, 0:1], in_=_f32(wcols[:, dj : dj + 1]))
            nc.vector.tensor_copy(out=w1z[:, dj, 1:2], in_=zerof_col)

        def load(b):
            qaug = qb_pool.tile([128, nq, d + 2], F32R, tag="qaug", name="qaug")
            nc.gpsimd.dma_start(
                out=qaug[:, :, 0:d],
                in_=q_in[b].rearrange("(p s) d -> p s d", p=128).bitcast(F32R),
            )
            cb = cb_pool.tile([128, nt, d], F32R, tag="cb", name="cb")
            cv = c_in[b].rearrange("(p s) d -> p s d", p=128).bitcast(F32R)
            for g in range(4):
                sl = slice(g * (nt // 4), (g + 1) * (nt // 4))
                nc.gpsimd.dma_start(out=cb[:, sl, :], in_=cv[:, sl, :])
            return cb, qaug

        def compute(b, cb, qaug):
            # ---- per-batch Q prep ----
            nc.vector.tensor_copy(out=qaug[:, :, d : d + 2], in_=oz[:, 0:nq, :])

            qt = qb_pool.tile([128, nd, tq], F32R, tag="qt")
            psq = psC.tile([128, nd * tq], F32, tag="psC")
            for dj in range(nd):
                for qi in range(nq):
                    nc.tensor.transpose(
                        psq[:, dj * tq + qi * 128 : dj * tq + (qi + 1) * 128],
                        _f32(qaug[:, qi, dj * 128 : (dj + 1) * 128]),
                        ident,
                    )
            nc.scalar.copy(out=qt, in_=psq)

            # q2 row = w2^T @ Q^T -> [1, tq]; pad col tq with 0
            psq2 = psU.tile([1, tq], F32, tag="psU")
            for dj in range(nd):
                nc.tensor.matmul(
                    psq2,
                    wcols[:, nd + dj : nd + dj + 1],
                    qt[:, dj, :],
                    start=(dj == 0),
                    stop=(dj == nd - 1),
                )
            q2row = pb_pool.tile([1, tq], F32R, tag="q2row")
            nc.vector.tensor_copy(out=q2row, in_=psq2)

            # q2col[p, qi] = q2[2p+qi]  (bias operand for the E^T exp);
            # a K=1 matmul broadcasts the q2 row chunk onto partitions
            psqc = psC.tile([128, 2 * nq], F32, tag="psC")
            for qi in range(nq):
                nc.tensor.matmul(
                    psqc[:, qi * 2 : (qi + 1) * 2],
                    q2row[:, qi * 128 : (qi + 1) * 128],
                    ones128[:, 0:2],
                    start=True,
                    stop=True,
                )
            q2col = pb_pool.tile([128, nq], F32, tag="q2col")
            for qi in range(nq):
                nc.vector.tensor_copy(
                    out=q2col[:, qi : qi + 1], in_=psqc[:, qi * 2 : qi * 2 + 1]
                )

            # qta[:, dj, :] = w3-scaled Q^T chunk
            qta = qb_pool.tile([128, nd, tq], F32R, tag="qta")
            for dj in range(nd):
                nc.vector.tensor_scalar_mul(
                    out=qta[:, dj, :],
                    in0=_f32(qt[:, dj, :]),
                    scalar1=_f32(wcols[:, 2 * nd + dj : 2 * nd + dj + 1]),
                )

            mfull = pb_pool.tile([128, nt], F32, tag="mfull")
            e2 = pb_pool.tile([128, nt], F32R, tag="e2")
            psh = psH.tile([1, d], F32, tag="psH")
            psc1 = psU.tile([128, nt, 2], F32, tag="psz", bufs=1)
            outv = out_e[b].rearrange("(p s) d -> p s d", p=128)
            # G's first d columns are C verbatim: one 2 MB DMA straight from
            # cb, queued at batch start so the out direction is busy from t=0
            if emit_out:
                nc.sync.dma_start(out=outv[:, :, 0:d], in_=_f32(cb))

            # ---- phase A: t-tile pairs ----
            for pj in range(npair):
                psc2 = psC.tile([128, nd * 256], F32, tag="psC")
                for jj in range(2):
                    s = 2 * pj + jj
                    for dj in range(nd):
                        nc.tensor.transpose(
                            psc2[:, dj * 256 + jj * 128 : dj * 256 + (jj + 1) * 128],
                            _f32(cb[:, s, dj * 128 : (dj + 1) * 128]),
                            ident,
                        )
                ct2 = ct_pool.tile([128, nd * 256], F32R, tag="ct2")
                if pj % 2 == 0:
                    nc.scalar.copy(out=ct2, in_=psc2)
                else:
                    nc.vector.tensor_copy(out=ct2, in_=psc2)

                # S for the PAIR in one bank [128, 2, tq] (row-max only);
                # c1 = C.w1 accumulates separately into psc1
                pss = ps2k.tile([128, 2, tq], F32, tag="ps2k")
                for jj in range(2):
                    s = 2 * pj + jj
                    for dj in range(nd):
                        nc.tensor.matmul(
                            pss[:, jj, :],
                            ct2[:, dj * 256 + jj * 128 : dj * 256 + (jj + 1) * 128],
                            qta[:, dj, :],
                            start=(jj == 0 and dj == 0),
                            stop=False,
                        )
                    nc.tensor.matmul(
                        pss[:, jj, :], ones128, q2row, start=False, stop=(jj == 1)
                    )
                    for dj in range(nd):
                        nc.tensor.matmul(
                            psc1[:, s, :],
                            ct2[:, dj * 256 + jj * 128 : dj * 256 + (jj + 1) * 128],
                            w1z[:, dj, :],
                            start=(dj == 0),
                            stop=(dj == nd - 1),
                        )
                nc.vector.reduce_max(
                    out=mfull[:, 2 * pj : 2 * pj + 2], in_=pss, axis=AX.X
                )

                # S'^T for the pair: psT2 layout [qi, (jj t)]; q2 added via
                # the exp bias (q on partitions here)
                psT2 = ps2k.tile([128, nq * 256], F32, tag="ps2k")
                for qi in range(nq):
                    sl = slice(qi * 256, (qi + 1) * 256)
                    for dj in range(nd):
                        nc.tensor.matmul(
                            psT2[:, sl],
                            qta[:, dj, qi * 128 : (qi + 1) * 128],
                            ct2[:, dj * 256 : (dj + 1) * 256],
                            start=(dj == 0),
                            stop=(dj == nd - 1),
                        )
                et2 = et_pool.tile([128, nq * 256], F32R, tag="et2")
                for qi in range(nq):
                    sl = slice(qi * 256, (qi + 1) * 256)
                    nc.scalar.activation(
                        out=et2[:, sl],
                        in_=psT2[:, sl],
                        func=AF.Exp,
                        bias=q2col[:, qi : qi + 1],
                    )

                # [U_raw | Z] = E @ [Q | 1]; stream [C | U | C*U] out per tile
                for jj in range(2):
                    s = 2 * pj + jj
                    psu = psU.tile([128, d + 2], F32, tag="psU")
                    for qi in range(nq):
                        nc.tensor.matmul(
                            psu,
                            et2[:, qi * 256 + jj * 128 : qi * 256 + (jj + 1) * 128],
                            qaug[:, qi, :],
                            start=(qi == 0),
                            stop=(qi == nq - 1),
                        )
                    rz = sm_pool.tile([128, 1], F32, tag="rz")
                    nc.vector.reciprocal(out=rz, in_=psu[:, d : d + 1])
                    gu = gu_pool.tile([128, 2 * d], F32, tag="gu")
                    nc.scalar.mul(gu[:, 0:d], psu[:, 0:d], rz)
                    nc.vector.tensor_mul(
                        out=gu[:, d : 2 * d],
                        in0=_f32(cb[:, s, :]),
                        in1=gu[:, 0:d],
                    )
                    if emit_out:
                        nc.sync.dma_start(out=outv[:, s, d : 3 * d], in_=gu)

            # ---- phase A epilogue: m = rowmax + c1, e2, h accumulation ----
            nc.vector.tensor_add(out=mfull, in0=mfull, in1=psc1[:, :, 0])
            nc.scalar.activation(out=e2, in_=mfull, func=AF.Exp)
            for s in range(nt):
                nc.tensor.matmul(
                    psh,
                    e2[:, s : s + 1],
                    cb[:, s, :],
                    start=(s == 0),
                    stop=(s == nt - 1),
                )

            # ---- phase B: Zb, h, hb; C*h multiplied and DMA'd on GpSimd ----
            psz = psU.tile([1, nt], F32, tag="psz", bufs=1)
            nc.tensor.matmul(psz, ones2r[:, 0:1], e2, start=True, stop=True)
            zsum = sm_pool.tile([1, 1], F32, tag="zsum")
            nc.vector.reduce_sum(out=zsum, in_=psz, axis=AX.X)
            zb = sm_pool.tile([1, 1], F32, tag="zb")
            nc.vector.reciprocal(out=zb, in_=zsum)
            hrow = pb_pool.tile([1, d], F32R, tag="hrow")
            nc.vector.tensor_scalar_mul(out=hrow, in0=psh, scalar1=zb)
            pshb = psU.tile([128, d], F32, tag="psz", bufs=1)
            nc.tensor.matmul(pshb, ones128, hrow, start=True, stop=True)
            hb = pb_pool.tile([128, d], F32, tag="hb")
            nc.scalar.copy(out=hb, in_=pshb)
            for s in range(nt):
                g4 = g4_pool.tile([128, d], F32, tag="g4")
                if s % 2 == 0:
                    nc.gpsimd.tensor_mul(out=g4, in0=_f32(cb[:, s, :]), in1=hb)
                    if emit_out:
                        nc.gpsimd.dma_start(out=outv[:, s, 3 * d : 4 * d], in_=g4)
                else:
                    nc.vector.tensor_mul(out=g4, in0=_f32(cb[:, s, :]), in1=hb)
                    if emit_out:
                        nc.sync.dma_start(out=outv[:, s, 3 * d : 4 * d], in_=g4)

        def body():
            tiles = load(0)
            for b in range(bpc):
                nxt = load(b + 1) if b + 1 < bpc else None
                compute(b, *tiles)
                tiles = nxt

        if reps is None:
            body()
        else:
            with tc.For_i(0, reps, 1):
                body()

    return nc


_NC_CACHE = {}


def _get_nc(bpc=BPC, tcl=TC, tq=TQ, d=D):
    key = (bpc, tcl, tq, d)
    if key not in _NC_CACHE:
        _NC_CACHE[key] = build_nc(*key)
    return _NC_CACHE[key]


def _run(context_emb, query_emb, w, trace=False, **spmd_kwargs):
    from concourse.bass_utils import run_bass_kernel_spmd

    context_emb = np.ascontiguousarray(np.asarray(context_emb, dtype=np.float32))
    query_emb = np.ascontiguousarray(np.asarray(query_emb, dtype=np.float32))
    w = np.ascontiguousarray(np.asarray(w, dtype=np.float32))

    nc = _get_nc()
    if not nc.is_finalized():
        nc.finalize()
    in_maps = []
    for c in range(N_CORES):
        sl = slice(c * BPC, (c + 1) * BPC)
        in_maps.append(
            {
                "context_emb": np.ascontiguousarray(context_emb[sl]),
                "query_emb": np.ascontiguousarray(query_emb[sl]),
                "w": w,
            }
        )
    res = run_bass_kernel_spmd(
        nc, in_maps, core_ids=list(range(N_CORES)), trace=trace, **spmd_kwargs
    )
    out = np.concatenate([r["out"] for r in res.results], axis=0)
    return out, res


def kernel(context_emb, query_emb, w):
    out, _ = _run(context_emb, query_emb, w, trace=False)
    return out


# revision 27
# speedup vs baseline: 1.4502x; 1.4420x over previous
"""Trainium2 Bass kernel: AttentionFlow layer (BiDAF-style), data-parallel over batch.

Reference semantics (per batch b, shapes C[Tc,d], Q[Tq,d], w[3d]):
    w1, w2, w3 = w[:d], w[d:2d], w[2d:]
    S[t,q]  = C[t].w1 + Q[q].w2 + (C[t]*w3).Q[q]
    P       = softmax_q(S)
    bt      = softmax_t(max_q S)
    U       = P @ Q
    h       = bt @ C
    G       = concat(C, U, C*U, C*h[None,:])   # [Tc, 4d]

On-chip identities used:
  - softmax_q(S) drops the C.w1 term (constant along q):  P = E/Z with
    E = exp(dot + q2), dot[t,q] = (C*w3)[t].Q[q], q2[q] = Q[q].w2.
    |dot + q2| <~ 5 so exp is fp32-safe without max subtraction.
  - max_q S = c1 + max_q(dot + q2) with c1 = C.w1 (extra w1 column on the
    S-matmul rhs; q2 row added with a K=1 ones-row matmul). S is only used
    for the row-max.
  - E^T = exp(S'^T + q2) with S'^T = qta^T @ C^T computed per t-tile PAIR;
    the q2 add rides the exp's per-partition bias operand (q on partitions
    in this orientation), saving the ones-rhs matmuls.
  - [U_raw | Z] = E @ [Q | 1]  (ones column appended to Q).
  - h_raw accumulated per tile; Zb via a ones-lhsT matmul over e2 + reduce.
  - Matmuls run as float32r; f32r SBUF operands must be PRODUCED as f32r.

Engine placement (keeps the DMA queues fed and compute queues unblocked):
  - out G[:, 0:3d] per tile -> SP-issued HWDGE DMA (SP is otherwise idle, so
    its trigger waits don't block compute).
  - out G[:, 3d:4d] (C*h) per tile: multiplied on GpSimd and DMA'd from
    GpSimd right after -- the trigger's wait is same-engine, i.e. free.
  - C/Q loads are software-prefetched one batch ahead on GpSimd.
  - PSUM->SBUF C^T copies alternate Scalar/Vector; U-mul on Scalar,
    C*U on Vector, row-max chain on Vector.

DMA layout: p-major row mapping (row = p*16 + s for C/G, row = 2p + s for
Q): the C load is one 2 MB DMA with 16 KB/partition descriptors; per-tile G
writes have one contiguous 3 KB (resp. 1 KB) descriptor per partition. The
t/q permutation is internal: all math is row-local or full reductions, so
the same mapping on loads and stores cancels it.
"""

import numpy as np

import concourse.bass as bass
import concourse.bacc as bacc
import concourse.mybir as mybir
import concourse.tile as tile
from contextlib import ExitStack
from concourse.masks import make_identity

F32 = mybir.dt.float32
F32R = mybir.dt.float32r
AX = mybir.AxisListType
AF = mybir.ActivationFunctionType

B, TC, TQ, D = 32, 2048, 256, 256
N_CORES = 8
BPC = B // N_CORES


def _f32(ap):
    """Plain-fp32 view of a float32r tile for non-matmul readers."""
    return ap.bitcast(F32)


def build_nc(bpc=BPC, tcl=TC, tq=TQ, d=D, reps=None, emit_out=True):
    nt = tcl // 128  # t-tiles per batch; tile s holds rows {p*nt + s}
    nd = d // 128    # K-chunks over d
    nq = tq // 128   # K-chunks over q
    npair = nt // 2
    assert nt % 2 == 0

    nc = bacc.Bacc(None, debug=False, target_bir_lowering=False)
    c_in = nc.declare_dram_parameter("context_emb", [bpc, tcl, d], F32, isOutput=False)
    q_in = nc.declare_dram_parameter("query_emb", [bpc, tq, d], F32, isOutput=False)
    w_in = nc.declare_dram_parameter("w", [3 * d], F32, isOutput=False)
    out_e = nc.declare_dram_parameter("out", [bpc, tcl, 4 * d], F32, isOutput=True)

    with tile.TileContext(nc) as tc, ExitStack() as ctx:
        singles = ctx.enter_context(tc.tile_pool(name="singles", bufs=1))
        cb_pool = ctx.enter_context(tc.tile_pool(name="cb", bufs=3))
        qb_pool = ctx.enter_context(tc.tile_pool(name="qb", bufs=3))
        pb_pool = ctx.enter_context(tc.tile_pool(name="pb", bufs=2))
        ct_pool = ctx.enter_context(tc.tile_pool(name="ct", bufs=4))
        et_pool = ctx.enter_context(tc.tile_pool(name="et", bufs=4))
        gu_pool = ctx.enter_context(tc.tile_pool(name="gu", bufs=8))
        g4_pool = ctx.enter_context(tc.tile_pool(name="g4", bufs=8))
        sm_pool = ctx.enter_context(tc.tile_pool(name="sm", bufs=6))
        ps2k = ctx.enter_context(tc.tile_pool(name="ps2k", bufs=2, space="PSUM"))
        psH = ctx.enter_context(tc.tile_pool(name="psH", bufs=1, space="PSUM"))
        psU = ctx.enter_context(tc.tile_pool(name="psU", bufs=2, space="PSUM"))
        psC = ctx.enter_context(tc.tile_pool(name="psC", bufs=2, space="PSUM"))

        ident = singles.tile([128, 128], F32, tag="ident")
        make_identity(nc, ident)
        onesf_col = singles.tile([128, 8], F32, tag="onesf_col")
        nc.vector.memset(onesf_col, 1.0)
        # oz[:, s, :] = [1.0, 0.0] -- pad columns for the even-N f32r matmuls
        oz = singles.tile([128, 8, 2], F32, tag="oz")
        nc.vector.memset(oz[:, :, 0:1], 1.0)
        nc.vector.memset(oz[:, :, 1:2], 0.0)
        zerof_col = singles.tile([128, 1], F32, tag="zerof_col")
        nc.vector.memset(zerof_col, 0.0)
        onesf_row = singles.tile([1, 256], F32, tag="onesf_row")
        nc.vector.memset(onesf_row, 1.0)
        zerof = singles.tile([1, 1], F32, tag="zerof")
        nc.vector.memset(zerof, 0.0)
        ones128 = singles.tile([1, 128], F32R, tag="ones128")
        nc.vector.tensor_copy(out=ones128, in_=onesf_row[:, 0:128])
        ones2r = singles.tile([128, 2], F32R, tag="ones2r")
        nc.vector.tensor_copy(out=ones2r, in_=onesf_col[:, 0:2])
        # wcols[p, k] = w[k*128 + p]: chunk columns [w1 | w2 | w3].
        # w comes in as one contiguous row (single descriptor) and is spread
        # onto partitions with K=1 matmuls.
        wrow = singles.tile([1, 3 * d], F32R, tag="wrow")
        nc.gpsimd.dma_start(out=wrow, in_=w_in[:].rearrange("(a w) -> a w", a=1).bitcast(F32R))
        wcols = singles.tile([128, 3 * nd], F32R, tag="wcols")
        pswc = psC.tile([128, 2 * 3 * nd], F32, tag="psC")
        for k in range(3 * nd):
            nc.tensor.matmul(
                pswc[:, 2 * k : 2 * k + 2],
                wrow[:, k * 128 : (k + 1) * 128],
                ones128[:, 0:2],
                start=True,
                stop=True,
            )
        for k in range(3 * nd):
            nc.vector.tensor_copy(
                out=wcols[:, k : k + 1], in_=pswc[:, 2 * k : 2 * k + 1]
            )
        # w1z[:, dj, :] = [w1 chunk | 0] -- N=2 rhs for the c1 matmuls
        w1z = singles.tile([128, nd, 2], F32R, tag="w1z")
        for dj in range(nd):
            nc.vector.tensor_copy(out=w1z[:, dj, 0:1], in_=_f32(wcols[:, dj : dj + 1]))
            nc.vector.tensor_copy(out=w1z[:, dj, 1:2], in_=zerof_col)

        def load(b):
            qaug = qb_pool.tile([128, nq, d + 2], F32R, tag="qaug", name="qaug")
            nc.gpsimd.dma_start(
                out=qaug[:, :, 0:d],
                in_=q_in[b].rearrange("(p s) d -> p s d", p=128).bitcast(F32R),
            )
            cb = cb_pool.tile([128, nt, d], F32R, tag="cb", name="cb")
            cv = c_in[b].rearrange("(p s) d -> p s d", p=128).bitcast(F32R)
            for g in range(4):
                sl = slice(g * (nt // 4), (g + 1) * (nt // 4))
                nc.gpsimd.dma_start(out=cb[:, sl, :], in_=cv[:, sl, :])
            return cb, qaug

        def compute(b, cb, qaug):
            # ---- per-batch Q prep ----
            nc.vector.tensor_copy(out=qaug[:, :, d : d + 2], in_=oz[:, 0:nq, :])

            qt = qb_pool.tile([128, nd, tq], F32R, tag="qt")
            psq = psC.tile([128, nd * tq], F32, tag="psC")
            for dj in range(nd):
                for qi in range(nq):
                    nc.tensor.transpose(
                        psq[:, dj * tq + qi * 128 : dj * tq + (qi + 1) * 128],
                        _f32(qaug[:, qi, dj * 128 : (dj + 1) * 128]),
                        ident,
                    )
            nc.scalar.copy(out=qt, in_=psq)

            # q2 row = w2^T @ Q^T -> [1, tq]; pad col tq with 0
            psq2 = psU.tile([1, tq], F32, tag="psU")
            for dj in range(nd):
                nc.tensor.matmul(
                    psq2,
                    wcols[:, nd + dj : nd + dj + 1],
                    qt[:, dj, :],
                    start=(dj == 0),
                    stop=(dj == nd - 1),
                )
            q2row = pb_pool.tile([1, tq], F32R, tag="q2row")
            nc.vector.tensor_copy(out=q2row, in_=psq2)

            # q2col[p, qi] = q2[2p+qi]  (bias operand for the E^T exp);
            # a K=1 matmul broadcasts the q2 row chunk onto partitions
            psqc = psC.tile([128, 2 * nq], F32, tag="psC")
            for qi in range(nq):
                nc.tensor.matmul(
                    psqc[:, qi * 2 : (qi + 1) * 2],
                    q2row[:, qi * 128 : (qi + 1) * 128],
                    ones128[:, 0:2],
                    start=True,
                    stop=True,
                )
            q2col = pb_pool.tile([128, nq], F32, tag="q2col")
            for qi in range(nq):
                nc.vector.tensor_copy(
                    out=q2col[:, qi : qi + 1], in_=psqc[:, qi * 2 : qi * 2 + 1]
                )

            # qta[:, dj, :] = w3-scaled Q^T chunk
            qta = qb_pool.tile([128, nd, tq], F32R, tag="qta")
            for dj in range(nd):
                nc.vector.tensor_scalar_mul(
                    out=qta[:, dj, :],
                    in0=_f32(qt[:, dj, :]),
                    scalar1=_f32(wcols[:, 2 * nd + dj : 2 * nd + dj + 1]),
                )

            mfull = pb_pool.tile([128, nt], F32, tag="mfull")
            e2 = pb_pool.tile([128, nt], F32R, tag="e2")
            psh = psH.tile([1, d], F32, tag="psH")
            psc1 = psU.tile([128, nt, 2], F32, tag="psz", bufs=1)
            outv = out_e[b].rearrange("(p s) d -> p s d", p=128)
            # G's first d columns are C verbatim: one 2 MB DMA straight from
            # cb, queued at batch start so the out direction is busy from t=0
            if emit_out:
                nc.sync.dma_start(out=outv[:, :, 0:d], in_=_f32(cb))

            # ---- phase A: t-tile pairs ----
            for pj in range(npair):
                psc2 = psC.tile([128, nd * 256], F32, tag="psC")
                for jj in range(2):
                    s = 2 * pj + jj
                    for dj in range(nd):
                        nc.tensor.transpose(
                            psc2[:, dj * 256 + jj * 128 : dj * 256 + (jj + 1) * 128],
                            _f32(cb[:, s, dj * 128 : (dj + 1) * 128]),
                            ident,
                        )
                ct2 = ct_pool.tile([128, nd * 256], F32R, tag="ct2")
                if pj % 2 == 0:
                    nc.scalar.copy(out=ct2, in_=psc2)
                else:
                    nc.vector.tensor_copy(out=ct2, in_=psc2)

                # S for the PAIR in one bank [128, 2, tq] (row-max only);
                # c1 = C.w1 accumulates separately into psc1
                pss = ps2k.tile([128, 2, tq], F32, tag="ps2k")
                for jj in range(2):
                    s = 2 * pj + jj
                    for dj in range(nd):
                        nc.tensor.matmul(
                            pss[:, jj, :],
                            ct2[:, dj * 256 + jj * 128 : dj * 256 + (jj + 1) * 128],
                            qta[:, dj, :],
                            start=(jj == 0 and dj == 0),
                            stop=False,
                        )
                    nc.tensor.matmul(
                        pss[:, jj, :], ones128, q2row, start=False, stop=(jj == 1)
                    )
                    for dj in range(nd):
                        nc.tensor.matmul(
                            psc1[:, s, :],
                            ct2[:, dj * 256 + jj * 128 : dj * 256 + (jj + 1) * 128],
                            w1z[:, dj, :],
                            start=(dj == 0),
                            stop=(dj == nd - 1),
                        )
                nc.vector.reduce_max(
                    out=mfull[:, 2 * pj : 2 * pj + 2], in_=pss, axis=AX.X
                )

                # S'^T for the pair: psT2 layout [qi, (jj t)]; q2 added via
                # the exp bias (q on partitions here)
                psT2 = ps2k.tile([128, nq * 256], F32, tag="ps2k")
                for qi in range(nq):
                    sl = slice(qi * 256, (qi + 1) * 256)
                    for dj in range(nd):
                        nc.tensor.matmul(
                            psT2[:, sl],
                            qta[:, dj, qi * 128 : (qi + 1) * 128],
                            ct2[:, dj * 256 : (dj + 1) * 256],
                            start=(dj == 0),
                            stop=(dj == nd - 1),
                        )
                et2 = et_pool.tile([128, nq * 256], F32R, tag="et2")
                for qi in range(nq):
                    sl = slice(qi * 256, (qi + 1) * 256)
                    nc.scalar.activation(
                        out=et2[:, sl],
                        in_=psT2[:, sl],
                        func=AF.Exp,
                        bias=q2col[:, qi : qi + 1],
                    )

                # [U_raw | Z] = E @ [Q | 1]; stream [C | U | C*U] out per tile
                for jj in range(2):
                    s = 2 * pj + jj
                    psu = psU.tile([128, d + 2], F32, tag="psU")
                    for qi in range(nq):
                        nc.tensor.matmul(
                            psu,
                            et2[:, qi * 256 + jj * 128 : qi * 256 + (jj + 1) * 128],
                            qaug[:, qi, :],
                            start=(qi == 0),
                            stop=(qi == nq - 1),
                        )
                    rz = sm_pool.tile([128, 1], F32, tag="rz")
                    nc.vector.reciprocal(out=rz, in_=psu[:, d : d + 1])
                    gu = gu_pool.tile([128, 2 * d], F32, tag="gu")
                    nc.scalar.mul(gu[:, 0:d], psu[:, 0:d], rz)
                    nc.vector.tensor_mul(
                        out=gu[:, d : 2 * d],
                        in0=_f32(cb[:, s, :]),
                        in1=gu[:, 0:d],
                    )
                    if emit_out:
                        nc.sync.dma_start(out=outv[:, s, d : 3 * d], in_=gu)

            # ---- phase A epilogue: m = rowmax + c1, e2, h accumulation ----
            nc.vector.tensor_add(out=mfull, in0=mfull, in1=psc1[:, :, 0])
            nc.scalar.activation(out=e2, in_=mfull, func=AF.Exp)
            for s in range(nt):
                nc.tensor.matmul(
                    psh,
                    e2[:, s : s + 1],
                    cb[:, s, :],
                    start=(s == 0),
                    stop=(s == nt - 1),
                )

            # ---- phase B: Zb, h, hb; C*h multiplied and DMA'd on GpSimd ----
            psz = psU.tile([1, nt], F32, tag="psz", bufs=1)
            nc.tensor.matmul(psz, ones2r[:, 0:1], e2, start=True, stop=True)
            zsum = sm_pool.tile([1, 1], F32, tag="zsum")
            nc.vector.reduce_sum(out=zsum, in_=psz, axis=AX.X)
            zb = sm_pool.tile([1, 1], F32, tag="zb")
            nc.vector.reciprocal(out=zb, in_=zsum)
            hrow = pb_pool.tile([1, d], F32R, tag="hrow")
            nc.vector.tensor_scalar_mul(out=hrow, in0=psh, scalar1=zb)
            pshb = psU.tile([128, d], F32, tag="psz", bufs=1)
            nc.tensor.matmul(pshb, ones128, hrow, start=True, stop=True)
            hb = pb_pool.tile([128, d], F32, tag="hb")
            nc.scalar.copy(out=hb, in_=pshb)
            for s in range(nt):
                g4 = g4_pool.tile([128, d], F32, tag="g4")
                if s % 2 == 0:
                    nc.gpsimd.tensor_mul(out=g4, in0=_f32(cb[:, s, :]), in1=hb)
                    if emit_out:
                        nc.gpsimd.dma_start(out=outv[:, s, 3 * d : 4 * d], in_=g4)
                else:
                    nc.vector.tensor_mul(out=g4, in0=_f32(cb[:, s, :]), in1=hb)
                    if emit_out:
                        nc.sync.dma_start(out=outv[:, s, 3 * d : 4 * d], in_=g4)

        def body():
            tiles = load(0)
            for b in range(bpc):
                nxt = load(b + 1) if b + 1 < bpc else None
                compute(b, *tiles)
                tiles = nxt

        if reps is None:
            body()
        else:
            with tc.For_i(0, reps, 1):
                body()

    return nc


_NC_CACHE = {}


def _get_nc(bpc=BPC, tcl=TC, tq=TQ, d=D):
    key = (bpc, tcl, tq, d)
    if key not in _NC_CACHE:
        _NC_CACHE[key] = build_nc(*key)
    return _NC_CACHE[key]


def _run(context_emb, query_emb, w, trace=False, **spmd_kwargs):
    from concourse.bass_utils import run_bass_kernel_spmd

    context_emb = np.ascontiguousarray(np.asarray(context_emb, dtype=np.float32))
    query_emb = np.ascontiguousarray(np.asarray(query_emb, dtype=np.float32))
    w = np.ascontiguousarray(np.asarray(w, dtype=np.float32))

    nc = _get_nc()
    if not nc.is_finalized():
        nc.finalize()
    in_maps = []
    for c in range(N_CORES):
        sl = slice(c * BPC, (c + 1) * BPC)
        in_maps.append(
            {
                "context_emb": np.ascontiguousarray(context_emb[sl]),
                "query_emb": np.ascontiguousarray(query_emb[sl]),
                "w": w,
            }
        )
    res = run_bass_kernel_spmd(
        nc, in_maps, core_ids=list(range(N_CORES)), trace=trace, **spmd_kwargs
    )
    out = np.concatenate([r["out"] for r in res.results], axis=0)
    return out, res


def kernel(context_emb, query_emb, w):
    out, _ = _run(context_emb, query_emb, w, trace=False)
    return out
